# revision 103
# baseline (speedup 1.0000x reference)
"""Trainium2 Bass kernel for nn_AlignLoss3 (anchor-alignment InfoNCE-style loss).

Math reduction
--------------
reference:
    label = argmax(Y, axis=1)                       # (N,) in 0..6
    A = l2norm(anchors)[label]; B = l2norm(X)
    logits = B @ A.T / tau                          # (N, N)
    loss = mean(logsumexp(logits, 1) - diag(logits))

Since logits[i, j] = B[i] . a_norm[label[j]] / tau depends on j only through
label[j] (7 classes), define S = B @ a_norm.T / tau  (N x 7) and the class
histogram cnt[c] = #{j : label[j] = c}.  Then
    logsumexp(logits[i,:]) = log( sum_c cnt[c] * exp(S[i,c]) )
    diag[i]               = S[i, label[i]]
so the N x N matmul collapses to an N x 7 one: the kernel is memory-bound on
reading X (16 MB).

Device / host split
-------------------
The device does only the memory-bound heavy lifting: per row i it returns the
raw (unnormalized, bf16) dot products R[i, c] = x_i . anchors_c and the raw
sum of squares ss[i] = |x_i|^2, packed per core as one [128, 64] f32 tile
(cols 7j..7j+6 = R for tile j, col 56+j = ss).  Y is never touched on device.
The host then computes, in float64 over 8192x7 values (trivial cost):
    S = R / (sqrt(ss)_i * |anchors_c| * tau);  loss = mean(log(exp(S) @ cnt) - S[i, lab_i])
which reproduces the reference to ~1e-6 (R/ss carry only bf16 rounding).

Sharding (8 cores)
------------------
Row-parallel: core k owns rows 1024k..1024(k+1).  X tile j = shard rows
{8p + j} at partition p.  The host gathers the 8 per-core [128, 64] tiles
and finishes the (embarrassingly cheap) loss reduction.

Per-core pipeline
-----------------
* a tiny first DMA warms the SP queue's pipeline (followers' completion
  semaphores then arrive ~350ns after transfer instead of ~1.7us); tiles
  0-5 and 7 stream on SP, tile 6 rides the otherwise-idle ACT HWDGE queue,
  anchors ride the Pool SWDGE ring (cast to bf16 on Pool so the ancT
  transposes carry one consolidated sync wait).
* per X tile: Pool casts f32->bf16, PE transposes 4 chunks into PSUM, one
  DVE 2x copy returns them to SBUF, four bf16 matmuls accumulate R in f32
  PSUM, a DVE copy stashes R into the result tile; ACT Squares (accum_out)
  produce ss straight off the f32 tile.  Tail tiles' copies/stashes are
  stream-prioritized.
* the out store is one SP HWDGE DMA; its cross-engine sync waits are
  re-homed onto SP nops (walrus allows one sync wait per instruction), and
  the exit drain re-homes its waits the same way with one barrier dropped.
"""

import numpy as np

import concourse.bass as bass
import concourse.tile as tile
from concourse import mybir
from concourse.bass_utils import run_bass_kernel_spmd
from concourse.masks import make_identity

N, D, C = 8192, 512, 7
NCORES = 8
P = 128
RPC = N // NCORES            # rows per core = 1024
JT = RPC // P                # X tiles per core = 8
TAU = 0.07
F32 = mybir.dt.float32
BF16 = mybir.dt.bfloat16
DCH = D // P                 # d-chunks = 4
AF = mybir.ActivationFunctionType
ALU = mybir.AluOpType
AX = mybir.AxisListType
OW = JT * C + JT             # out width = 56 R cols + 8 ss cols = 64


class SplitWaitTileContext(tile.TileContext):
    """TileContext whose exit drain never carries more than one sync wait.

    This container's walrus build rejects any instruction encoding more than
    one sync-wait command.  Tile's exit drain waits on every proc's final
    tick.  Pre-drain, emit one SP nop per pending wait — the SP sequencer is
    in-order, so by the time the real drain issues, the wait clock shows
    everything observed and the drain gets no waits.
    """

    def _drain_and_barrier(self, tick_clock, wait_clock):
        import bass_rust

        nc = self.nc
        nops = [nc.sync.nop(nofuse=True, hint=f"split_wait_{i}") for i in range(16)]

        drain_inst = nc.sync.drain()
        wait_clock.add_sem_waits(
            drain_inst.ins,
            bass_rust.ScopedClock({None: tick_clock.global_clock}),
        )
        si = drain_inst.ins.sync_info
        waits = list(si.on_wait) if si is not None else []
        if len(waits) > 1:
            assert len(waits) - 1 <= len(nops), "raise the split-wait nop count"
            si.on_wait = waits[-1:]
            for nop, w in zip(nops, waits[:-1]):
                nop.ins.sync_info = bass_rust.SyncInfo(on_wait=[w], on_update=[])

        nc.all_engine_barrier()
        assert self.sems is not None
        popped = nc._tile_sem_poison_stack.pop()
        assert popped is self._sem_poison
        nc.clear_and_free_semaphores(list(self.sems.allocated().values()))


def build_kernel() -> bass.Bass:
    nc = bass.Bass(num_swdge_queues=2)

    xs = nc.dram_tensor("xs", [RPC, D], F32, kind="ExternalInput")
    anc = nc.dram_tensor("anc", [C, D], F32, kind="ExternalInput")
    out = nc.dram_tensor("out", [P, OW], F32, kind="ExternalOutput")

    with SplitWaitTileContext(nc) as tc:
        with (
            tc.tile_pool(name="consts", bufs=1) as consts,
            tc.tile_pool(name="xpool", bufs=8) as xpool,
            tc.tile_pool(name="xbpool", bufs=8) as xbpool,
            tc.tile_pool(name="xtpool", bufs=8) as xtpool,
            tc.tile_pool(name="psum", bufs=3, space="PSUM") as psum,
            tc.tile_pool(name="psum_s", bufs=2, space="PSUM") as psum_s,
            tc.tile_pool(name="psum_a", bufs=1, space="PSUM") as psum_a,
        ):
            ident_bf = consts.tile([P, P], BF16)
            make_identity(nc, ident_bf[:])
            ones = consts.tile([P, 1], F32)
            nc.vector.memset(ones[:], 1.0)

            # result tile, written column-wise by the R stashes / Squares
            # and stored whole by one tail DMA
            res64 = consts.tile([P, OW], F32)

            # a tiny first DMA warms the SP queue's pipeline so x0's
            # completion comes ~350ns after its transfer instead of paying
            # the ~1.7us first-DMA latency
            warm_dma = consts.tile([1, 1], F32)
            nc.sync.dma_start(out=warm_dma[:], in_=anc[0:1, 0:1])

            # warm the activation table holding Square off the critical path
            # (the first table-based ACT op pays a 1283 ns table load)
            warm = consts.tile([1, 1], F32)
            nc.scalar.activation(out=warm[:], in_=ones[:1, :], func=AF.Square)

            # ---- X stream ----
            # Tiles 0-5 and 7 ride the warmed SP queue; tile 6 rides the
            # ACT HWDGE queue (idle but for the table warm) so the SP
            # stream — and with it tile 7, whose chain sets the kernel
            # tail — finishes one transfer earlier.  Anchors ride the idle
            # Pool SWDGE ring.
            anc_s = consts.tile([C, D], F32)
            nc.gpsimd.dma_start(out=anc_s[:], in_=anc[:])
            xs_r = xs[:].rearrange("(p j) d -> j p d", j=JT)
            x_tiles = []
            for j in range(JT):
                x_t = xpool.tile([P, D], F32)
                if j == 6:
                    nc.scalar.dma_start(out=x_t[:], in_=xs_r[j])
                else:
                    nc.sync.dma_start(out=x_t[:], in_=xs_r[j])
                x_tiles.append(x_t)

            # ---- anchors: cast bf16 on Pool (same engine as the identity
            # writer, so the transposes carry ONE consolidated sync wait —
            # the walrus build rejects multi-wait TR instructions), then
            # transpose to ancT (raw values; normalization on the host) ----
            anc_b = consts.tile([C, D], BF16)
            with tc.high_priority():
                nc.gpsimd.tensor_copy(out=anc_b[:], in_=anc_s[:])
            ancT = consts.tile([P, DCH, C], BF16)
            # pad the per-chunk stride to 8 to keep PSUM writes 4B-aligned
            ps_a = psum_a.tile([P, DCH, 8], BF16, tag="ps_anc")
            for t in range(DCH):
                nc.tensor.transpose(
                    ps_a[:, t, 0:C], anc_b[:, t * P:(t + 1) * P], ident_bf[:C, :C]
                )
            with tc.high_priority():
                nc.vector.tensor_copy(out=ancT[:], in_=ps_a[:, :, 0:C])

            for j in range(JT):
                x_t = x_tiles[j]

                # cast to bf16 on Pool for every tile: by the time tiles 6/7
                # land, Pool's cast chain has drained, so it runs them
                # immediately and in parallel with DVE's copy/stash tail
                xb = xbpool.tile([P, D], BF16)
                if j >= 6:
                    with tc.high_priority():
                        nc.gpsimd.tensor_copy(out=xb[:], in_=x_t[:])
                else:
                    nc.gpsimd.tensor_copy(out=xb[:], in_=x_t[:])

                # raw row sum of squares straight off the f32 tile into the
                # result column 56+j (ACT Squares; ACT has tail slack while
                # DVE's copy/stash chain sets the kernel end)
                sq_scr = xbpool.tile([P, D], F32, tag="sq_scr")
                nc.scalar.activation(
                    out=sq_scr[:], in_=x_t[:], func=AF.Square,
                    accum_out=res64[:, JT * C + j:JT * C + j + 1],
                )

                # PE: transpose 4 bf16 chunks into one PSUM tile
                ps_big = psum.tile([P, DCH, P], BF16)
                for t in range(DCH):
                    nc.tensor.transpose(
                        ps_big[:, t, :], xb[:, t * P:(t + 1) * P], ident_bf[:]
                    )
                # one DVE 2x copy PSUM -> SBUF; tiles 6/7 high-priority (they
                # gate the kernel tail)
                xT = xtpool.tile([P, DCH, P], BF16)
                if j >= 6:
                    with tc.high_priority():
                        nc.vector.tensor_copy(out=xT[:], in_=ps_big[:])
                else:
                    nc.vector.tensor_copy(out=xT[:], in_=ps_big[:])

                # R[rows, 7] = sum_t xT_t.T @ ancT_t (raw bf16 dots)
                ps_S = psum_s.tile([P, C], F32, tag="ps_small")
                for t in range(DCH):
                    nc.tensor.matmul(
                        ps_S[:], lhsT=xT[:, t, :], rhs=ancT[:, t, :],
                        start=(t == 0), stop=(t == DCH - 1),
                    )
                # stash R into the result columns 7j..7j+6
                if j == JT - 1:
                    with tc.high_priority():
                        nc.vector.tensor_copy(
                            out=res64[:, j * C:(j + 1) * C], in_=ps_S[:]
                        )
                else:
                    nc.vector.tensor_copy(
                        out=res64[:, j * C:(j + 1) * C], in_=ps_S[:]
                    )

            # out store: res64 is written by two engine streams (ACT Squares,
            # DVE stashes), but the walrus build allows only ONE sync wait
            # per instruction — the DMA's excess waits are re-homed onto the
            # SP nops below (the SplitWait drain trick: SP is in-order, so
            # by the time the DMA issues every re-homed wait has been
            # observed)
            out_nops = [
                nc.sync.nop(nofuse=True, hint=f"out_wait_{i}") for i in range(4)
            ]
            out_dma = nc.sync.dma_start(out=out[:], in_=res64[:])

    import bass_rust
    si = out_dma.ins.sync_info
    waits = list(si.on_wait) if si is not None else []
    if len(waits) > 1:
        assert len(waits) - 1 <= len(out_nops)
        si.on_wait = waits[-1:]
        for nop, w in zip(out_nops, waits[:-1]):
            nop.ins.sync_info = bass_rust.SyncInfo(on_wait=[w], on_update=[])

    return nc


_NC_CACHE: bass.Bass | None = None


def run_with_results(X, Y, anchors, **kwargs):
    """Run on all 8 cores; returns (loss, BassKernelResults)."""
    global _NC_CACHE
    if _NC_CACHE is None:
        _NC_CACHE = build_kernel()
    nc = _NC_CACHE

    X = np.ascontiguousarray(X, dtype=np.float32)
    Y = np.ascontiguousarray(Y, dtype=np.float32)
    anchors = np.ascontiguousarray(anchors, dtype=np.float32)

    in_maps = []
    for k in range(NCORES):
        in_maps.append({
            "xs": X[RPC * k:RPC * (k + 1)],
            "anc": anchors,
        })
    res = run_bass_kernel_spmd(nc, in_maps, core_ids=list(range(NCORES)), **kwargs)

    # unshard: core k's out[p, 7j+c] = R for global row 1024k + 8p + j,
    # out[p, 56+j] = |x|^2 for the same row
    R = np.empty((N, C), dtype=np.float64)
    ss = np.empty((N,), dtype=np.float64)
    for k in range(NCORES):
        o = np.asarray(res.results[k]["out"], dtype=np.float64)  # [128, 64]
        rows = RPC * k + 8 * np.arange(P)[:, None] + np.arange(JT)[None, :]
        R[rows.reshape(-1)] = o[:, :JT * C].reshape(P, JT, C).reshape(-1, C)
        ss[rows.reshape(-1)] = o[:, JT * C:].reshape(-1)

    # host epilogue (cheap): normalize, logsumexp over the 7-class collapse
    lab = np.argmax(Y, axis=1)
    cnt = np.bincount(lab, minlength=C).astype(np.float64)
    a_n = np.linalg.norm(anchors.astype(np.float64), axis=1)
    a_n = np.maximum(a_n, 1e-12)
    x_n = np.sqrt(np.maximum(ss, 0.0))
    x_n = np.maximum(x_n, 1e-12)
    S = R / (x_n[:, None] * a_n[None, :] * TAU)
    m = S.max(axis=1, keepdims=True)
    lse = np.log(np.exp(S - m) @ cnt) + m[:, 0]
    diag = S[np.arange(N), lab]
    loss = np.mean(lse - diag)
    return np.float32(loss), res


def kernel(X: np.ndarray, Y: np.ndarray, anchors: np.ndarray) -> np.ndarray:
    loss, _ = run_with_results(X, Y, anchors)
    return loss
